# revision 66
# baseline (speedup 1.0000x reference)
"""Adaptive-softmax loss (nn_AdaptiveLoss) on 8 trn2 NeuronCores.

Strategy: tensor-parallel over the vocab dimension, 8-way. Each core owns
1/8 of the shortlist head columns and 1/8 of each tail cluster's output
rows. Per core:

  - computes cluster hidden states h_g = x @ proj_g.T (replicated, small),
  - computes its slice of every group's logits with bf16 matmuls
    (out = lhsT.T @ rhs with lhsT = x.T / h.T tiles shipped pre-transposed),
  - exp()s the logits on the scalar engine (partial softmax denominators
    Z_g[b] accumulate on ACT for the tail clusters and on DVE for the
    shortlist head),
  - gathers exp(logit) at this core's share of the targets straight out of
    SBUF (gpsimd indirect_copy; indices + per-target loss weights are
    routed host-side as part of sharding),
  - reduces per-row partial loss statistics (numerator, weight sums, Z),
  - one 45KB AllReduce combines the statistics, then every core finishes
    the (cheap) log/normalize arithmetic identically and writes the scalar.

The full [B, VOCAB] log-prob matrix is never materialized anywhere.
"""

import sys

sys.path.insert(0, "/opt/trn_rl_repo")

from contextlib import ExitStack

import ml_dtypes
import numpy as np

import concourse.bass as bass  # noqa: F401  (engine types via nc.*)
import concourse.mybir as mybir
import concourse.tile as tile
from concourse import bacc
from concourse.bass_utils import run_bass_kernel_spmd

BF16 = ml_dtypes.bfloat16
F32 = mybir.dt.float32
BF16_DT = mybir.dt.bfloat16
U16 = mybir.dt.uint16

NCORES = 8
B, T, D = 1024, 128, 1024
VOCAB, SHORT = 100000, 10000
CL_SIZES = [10000, 20000, 40000, 20000]
CL_D = [512, 256, 128, 64]
SH_SHARD = SHORT // NCORES                      # 1250
CL_SHARD = [s // NCORES for s in CL_SIZES]      # 1250 2500 5000 2500
GRP_BOUNDS = [0, 10000, 20000, 40000, 80000, 100000]
GRP_SHARD = [SH_SHARD] + CL_SHARD

# per-core concatenated logits layout: [head | links(4) | c0 | c1 | c2 | c3 | pad]
OFF_HEAD = 0
OFF_LINK = SH_SHARD                              # 1250
OFF_CL = [1254, 2504, 5004, 10004]
GRP_OFF = [OFF_HEAD] + OFF_CL                    # per-group concat offset
CONCAT = OFF_CL[-1] + CL_SHARD[-1]               # 12504
CONCAT_PAD = 12544
# pad slots gather column 0 (always computed, finite); their tgid==0 makes
# the ownership mask zero the contribution.
PADIDX = 0
RT = 8                                           # row tiles of 128


# ----------------------------------------------------------------------------
# device kernel builder
# ----------------------------------------------------------------------------

_CACHE: dict[int, object] = {}


def _build(S: int):
    """Build + compile the SPMD kernel for padded slot count S (multiple of 16)."""
    if S in _CACHE:
        return _CACHE[S]
    SW = S // 16

    nc = bacc.Bacc("TRN2", target_bir_lowering=False, debug=False,
                   num_devices=NCORES)

    xt_d = nc.dram_tensor("xt", [D, B], BF16_DT, kind="ExternalInput")
    projt_d = nc.dram_tensor("projt", [D, sum(CL_D)], BF16_DT, kind="ExternalInput")
    whead_d = nc.dram_tensor("wheadt", [D, 1254], BF16_DT, kind="ExternalInput")
    wout_d = [
        nc.dram_tensor(f"wout{g}t", [CL_D[g], CL_SHARD[g]], BF16_DT,
                       kind="ExternalInput")
        for g in range(4)
    ]
    tix_d = nc.dram_tensor("tgtidx", [128, RT * SW], U16, kind="ExternalInput")
    tgid_d = nc.dram_tensor("tgid", [128, RT, S], BF16_DT, kind="ExternalInput")
    wm_d = nc.dram_tensor("wm", [128, RT, S], BF16_DT, kind="ExternalInput")
    out_d = nc.dram_tensor("out", [1, 1], F32, kind="ExternalOutput")

    EXP = mybir.ActivationFunctionType.Exp
    LN = mybir.ActivationFunctionType.Ln
    ADD = mybir.AluOpType.add
    SUB = mybir.AluOpType.subtract
    MULT = mybir.AluOpType.mult
    ISEQ = mybir.AluOpType.is_equal
    ISGE = mybir.AluOpType.is_ge
    AXX = mybir.AxisListType.X

    with tile.TileContext(nc) as tc, ExitStack() as ctx:
        sb = ctx.enter_context(tc.tile_pool(name="sb", bufs=1))
        big = ctx.enter_context(tc.tile_pool(name="big", bufs=3))
        ps = ctx.enter_context(tc.tile_pool(name="ps", bufs=2, space="PSUM"))
        dr = ctx.enter_context(tc.tile_pool(name="dr", bufs=1, space="DRAM"))

        # ---- persistent SBUF tensors ----
        xt_sb = sb.tile([128, 8, B], BF16_DT)          # x.T  [d, b] k-tiled
        whead_sb = sb.tile([128, 8, 1254], BF16_DT)
        wout_sb = [
            sb.tile([CL_D[g] if CL_D[g] < 128 else 128,
                     max(1, CL_D[g] // 128), CL_SHARD[g]], BF16_DT,
                    name=f"wout{g}_sb")
            for g in range(4)
        ]
        h_sb = sb.tile([128, 8, B], BF16_DT)           # h.T tiles (c3 rows 0:64 of ht7)
        tmpS = sb.tile([128, S], BF16_DT)
        tix_sb = sb.tile([128, RT * SW], U16)
        tgid_sb = sb.tile([128, RT, S], BF16_DT)
        vg3 = sb.tile([128, RT, S], BF16_DT)           # gathered exp(logit)
        wm_sb = sb.tile([128, RT, S], BF16_DT)         # (1-dp)*ownership
        logv3 = sb.tile([128, RT, S], BF16_DT)
        linkexp = sb.tile([128, RT, 4], F32)
        zscr = sb.tile([128, 1280], BF16_DT)
        zs7 = sb.tile([128, RT, 7], F32)   # Z partials per piece, c1/c2/c3
        # AR payload, 11 stats x 8 row-tiles:
        # q: 0 Zh, 1..4 Zc_g, 5 den, 6..9 Wg, 10 numraw
        pay = sb.tile([128, 88], F32)
        rsb = sb.tile([128, 88], F32)
        ones_sb = sb.tile([128, 1], F32)
        out_sb = sb.tile([1, 1], F32)

        pview = pay[:, :].rearrange("p (q r) -> p q r", q=11)
        rview = rsb[:, :].rearrange("p (q r) -> p q r", q=11)

        # ---- input DMAs (order matters: compute-critical tensors first;
        # xt/projt interleaved per k-tile so the h matmuls start early) ----
        projt_sb = big.tile([128, 8 * sum(CL_D)], BF16_DT, tag="big")
        pj = projt_sb[:, :].rearrange("p (k c) -> p k c", k=8)
        xt_r = xt_d.ap().rearrange("(k p) b -> p k b", p=128)
        pj_r = projt_d.ap().rearrange("(k p) c -> p k c", p=128)
        for k in range(8):
            nc.sync.dma_start(out=xt_sb[:, k, :], in_=xt_r[:, k, :])
            nc.sync.dma_start(out=pj[:, k, :], in_=pj_r[:, k, :])
        nc.sync.dma_start(out=whead_sb,
                          in_=whead_d.ap().rearrange("(k p) c -> p k c", p=128))
        for g in range(4):
            prt = min(128, CL_D[g])
            nc.sync.dma_start(
                out=wout_sb[g],
                in_=wout_d[g].ap().rearrange("(k p) c -> p k c", p=prt))
        nc.sync.dma_start(out=tix_sb, in_=tix_d[:])
        nc.sync.dma_start(out=tgid_sb, in_=tgid_d[:])
        nc.sync.dma_start(out=wm_sb, in_=wm_d[:])

        nc.vector.memset(ones_sb[:, :], 1.0)

        # ---- cluster hidden states h.T (all batch rows, computed locally —
        # a sharded h + AllGather is infeasible: the collectives entry
        # barrier isn't done until ~60-150us into the NEFF) ----
        HT_OFF = [0, 128, 256, 384, 512, 640, 768, 896]
        HT_M = [128, 128, 128, 128, 128, 128, 128, 64]
        for bc in range(2):
            for htile in range(2):
                pst = ps.tile([128, 2048], F32, tag="ps", name=f"hps_{bc}_{htile}")
                for hl in range(4):
                    ht = htile * 4 + hl
                    M = HT_M[ht]
                    for k in range(8):
                        nc.tensor.matmul(
                            pst[0:M, hl * 512:(hl + 1) * 512],
                            pj[:, k, HT_OFF[ht]:HT_OFF[ht] + M],
                            xt_sb[:, k, bc * 512:(bc + 1) * 512],
                            start=(k == 0), stop=(k == 7))
                for hl in range(4):
                    ht = htile * 4 + hl
                    M = HT_M[ht]
                    nc.vector.tensor_copy(
                        h_sb[0:M, ht, bc * 512:(bc + 1) * 512],
                        pst[0:M, hl * 512:(hl + 1) * 512])

        # ---- main loop: logits -> exp (+Z accumulate) -> gather, per row tile --
        # groups: (concat_off, width, n_ktiles, lhsT source, rhs tensor)
        # lhsT(k, rt): [K, 128] slice; rhs(k, a, w): [K, w] slice
        def lh_head(k, rt):
            return xt_sb[:, k, rt * 128:(rt + 1) * 128]

        def mk_lh_cl(ht0, prt):
            def f(k, rt):
                return h_sb[0:prt, ht0 + k, rt * 128:(rt + 1) * 128]
            return f

        def mk_rhs(tile_):
            def f(k, a, w):
                return tile_[:, k, a:a + w]
            return f

        GROUPS = [
            (0, 1254, 8, lh_head, mk_rhs(whead_sb)),
            (OFF_CL[0], 1250, 4, mk_lh_cl(0, 128), mk_rhs(wout_sb[0])),
            (OFF_CL[1], 2500, 2, mk_lh_cl(4, 128), mk_rhs(wout_sb[1])),
            (OFF_CL[2], 5000, 1, mk_lh_cl(6, 128), mk_rhs(wout_sb[2])),
            (OFF_CL[3], 2500, 1, mk_lh_cl(7, 64), mk_rhs(wout_sb[3])),
        ]

        for rt in range(RT):
            expb = big.tile([128, CONCAT_PAD], BF16_DT, tag="big",
                            name=f"expb_{rt}")
            zi = 0
            for gi, (goff, width, kt, lh, rh) in enumerate(GROUPS):
                # pieces of <=2048 cols; one ACT call per piece
                pieces = []
                a = 0
                while a < width:
                    w = min(2048, width - a)
                    pieces.append((a, w))
                    a += w
                for pi, (poff, pw) in enumerate(pieces):
                    pst = ps.tile([128, 2048], F32, tag="ps",
                                  name=f"ps_{rt}_{gi}_{pi}")
                    # k outer so the stationary operand is loaded once per
                    # k-tile instead of once per matmul
                    subs = []
                    a = 0
                    while a < pw:
                        w = min(512, pw - a)
                        subs.append((a, w))
                        a += w
                    for k in range(kt):
                        for a, w in subs:
                            nc.tensor.matmul(
                                pst[:, a:a + w],
                                lh(k, rt), rh(k, poff + a, w),
                                start=(k == 0), stop=(k == kt - 1))
                    if gi >= 2:
                        # cluster 1/2/3 pieces are single-group: fold the Z
                        # partial into the exp call on ACT
                        nc.scalar.activation(
                            expb[:, goff + poff:goff + poff + pw],
                            pst[:, 0:pw], EXP,
                            accum_out=zs7[:, rt, zi:zi + 1])
                        zi += 1
                    else:
                        nc.scalar.activation(
                            expb[:, goff + poff:goff + poff + pw],
                            pst[:, 0:pw], EXP)
            # head/c0 Z partial sums on DVE (head must exclude link columns)
            for q, (za, zw) in enumerate(
                    [(0, SH_SHARD), (OFF_CL[0], CL_SHARD[0])]):
                nc.vector.tensor_scalar(
                    zscr[:, 0:zw],
                    expb[:, za:za + zw], 1.0, 0.0, op0=MULT, op1=ADD,
                    accum_out=pview[:, q, rt:rt + 1])
            # raw exp(link logits) for the final combine (on ACT so the DVE
            # stats backlog cannot delay releasing this expbuf slot)
            nc.scalar.copy(linkexp[:, rt, :], expb[:, OFF_LINK:OFF_LINK + 4])
            # gather exp(logit) at this core's targets
            nc.gpsimd.indirect_copy(
                vg3[:, rt, :], expb[:, :],
                tix_sb[:, rt * SW:(rt + 1) * SW], True)
            # per-row-tile statistics that don't need log(v): den, W_g
            # (Wsl is never needed: den already equals Wsl + sum_g Wg)
            nc.vector.tensor_reduce(
                pview[:, 5, rt:rt + 1], wm_sb[:, rt, :], AXX, ADD)
            for gi in range(1, 5):
                nc.vector.scalar_tensor_tensor(
                    tmpS[:, :], tgid_sb[:, rt, :], float(gi + 1), wm_sb[:, rt, :],
                    op0=ISEQ, op1=MULT)
                nc.vector.tensor_reduce(
                    pview[:, 5 + gi, rt:rt + 1], tmpS[:, :], AXX, ADD)

        # combine the c1/c2/c3 Z piece-partials into the payload
        t8z = sb.tile([128, 8], F32)
        nc.vector.tensor_tensor(pview[:, 2, :], zs7[:, :, 0], zs7[:, :, 1], ADD)
        nc.vector.tensor_tensor(t8z[:, :], zs7[:, :, 2], zs7[:, :, 3], ADD)
        nc.vector.tensor_tensor(pview[:, 3, :], t8z[:, :], zs7[:, :, 4], ADD)
        nc.vector.tensor_tensor(pview[:, 4, :], zs7[:, :, 5], zs7[:, :, 6], ADD)

        # ---- numerator statistic (needs log of gathered exp) ----
        nc.scalar.activation(
            logv3[:, :, :].rearrange("p a b -> p (a b)"),
            vg3[:, :, :].rearrange("p a b -> p (a b)"), LN)
        tmp3 = vg3
        nc.vector.tensor_tensor(tmp3[:, :, :], logv3[:, :, :], wm_sb[:, :, :], MULT)
        nc.vector.tensor_reduce(pview[:, 10, :], tmp3[:, :, :], AXX, ADD)

        # ---- AllReduce the statistics ----
        cc_in = dr.tile([128, 88], F32)
        cc_out = dr.tile([128, 88], F32, addr_space="Shared")
        nc.sync.dma_start(out=cc_in, in_=pay[:, :])
        nc.gpsimd.collective_compute(
            "AllReduce", ADD,
            replica_groups=[list(range(NCORES))],
            ins=[cc_in.opt()], outs=[cc_out.opt()])
        nc.sync.dma_start(out=rsb, in_=cc_out)

        # ---- final combine (identical on every core) ----
        lsum = sb.tile([128, 8], F32)
        zf = sb.tile([128, 8], F32)
        lzh = sb.tile([128, 8], F32)
        lzc = sb.tile([128, 32], F32)
        llink = sb.tile([128, 32], F32)
        s8 = sb.tile([128, 8], F32)
        tA = sb.tile([128, 8], F32)
        num8 = sb.tile([128, 8], F32)
        rden = sb.tile([128, 8], F32)
        pcol = sb.tile([128, 1], F32)

        nc.vector.tensor_reduce(lsum[:, :], linkexp[:, :, :], AXX, ADD)
        nc.vector.tensor_tensor(zf[:, :], rview[:, 0, :], lsum[:, :], ADD)
        nc.scalar.activation(lzh[:, :], zf[:, :], LN)
        nc.scalar.activation(lzc[:, :], rsb[:, 8:40], LN)
        nc.scalar.activation(
            llink[:, :],
            linkexp[:, :, :].rearrange("p a b -> p (a b)"), LN)
        lzc3 = lzc[:, :].rearrange("p (g r) -> p g r", g=4)
        llink3 = llink[:, :].rearrange("p (r g) -> p r g", g=4)
        for g in range(4):
            nc.vector.tensor_tensor(tA[:, :], llink3[:, :, g], lzc3[:, g, :], SUB)
            if g == 0:
                nc.vector.tensor_tensor(s8[:, :], tA[:, :], rview[:, 6 + g, :], MULT)
            else:
                nc.vector.tensor_tensor(tA[:, :], tA[:, :], rview[:, 6 + g, :], MULT)
                nc.vector.tensor_tensor(s8[:, :], s8[:, :], tA[:, :], ADD)
        # num = numraw + s8 - den * logZh
        nc.vector.tensor_tensor(tA[:, :], rview[:, 5, :], lzh[:, :], MULT)
        nc.vector.tensor_tensor(num8[:, :], rview[:, 10, :], tA[:, :], SUB)
        nc.vector.tensor_tensor(num8[:, :], num8[:, :], s8[:, :], ADD)
        nc.vector.reciprocal(rden[:, :], rview[:, 5, :])
        nc.vector.tensor_tensor(num8[:, :], num8[:, :], rden[:, :], MULT)
        nc.vector.tensor_reduce(pcol[:, :], num8[:, :], AXX, ADD)
        psq = ps.tile([1, 1], F32, tag="ps")
        nc.tensor.matmul(psq[0:1, 0:1], pcol[:, 0:1], ones_sb[:, 0:1],
                         start=True, stop=True)
        nc.scalar.mul(out_sb[:, :], psq[0:1, 0:1], -1.0 / (B + 1e-5))
        nc.sync.dma_start(out=out_d[:], in_=out_sb)

    nc.compile()
    _CACHE[S] = nc
    return nc


# ----------------------------------------------------------------------------
# host-side sharding / index routing
# ----------------------------------------------------------------------------


def _shard_inputs(features, head_weight, projs, outs, discard_probs,
                  targets, target_mask):
    """Build the 8 per-core input maps. Returns (in_maps, S)."""
    xt = np.ascontiguousarray(features.T).astype(BF16)
    projt = np.concatenate([p.T for p in projs], axis=1).astype(BF16)
    linkT = head_weight[SHORT:SHORT + 4].T.astype(np.float32)

    tgt = np.asarray(targets).astype(np.int64).reshape(-1)
    msk = np.asarray(target_mask).astype(bool).reshape(-1)
    bb = np.repeat(np.arange(B, dtype=np.int64), T)

    grp = np.digitize(tgt, GRP_BOUNDS[1:-1])          # 0..4 (0 = shortlist)
    u = tgt - np.asarray(GRP_BOUNDS)[grp]
    shard = np.asarray(GRP_SHARD)[grp]
    core = u // shard
    jcat = u % shard + np.asarray(GRP_OFF)[grp]
    wval = (1.0 - discard_probs[tgt]).astype(np.float32)

    rt = bb >> 7
    gc = (bb >> 4) & 7

    # padded slots per (core, rt, gc)
    key_all = ((core * RT + rt) * 8 + gc).astype(np.int64)
    valid = msk
    counts = np.bincount(key_all[valid], minlength=NCORES * RT * 8)
    # multiple of 32 so each row-tile's wrapped idx slice stays 4B-aligned
    S = int(counts.max())
    S = ((S + 31) // 32) * 32

    in_maps = []
    for c in range(NCORES):
        sel = valid & (core == c)
        jj = jcat[sel]
        bsel = bb[sel]
        gsel = grp[sel]
        rts = rt[sel]
        gcs = gc[sel]
        ww = wval[sel]
        po = bsel & 15
        key = rts * 8 + gcs
        order = np.argsort(key, kind="stable")
        jj, bsel, gsel, rts, gcs, po, ww = (a[order] for a in
                                            (jj, bsel, gsel, rts, gcs, po, ww))
        key = key[order]
        # slot within each (rt, gc) bucket
        start_of = np.r_[0, np.flatnonzero(np.diff(key)) + 1]
        bucket_len = np.diff(np.r_[start_of, len(key)])
        slot = np.arange(len(key)) - np.repeat(start_of, bucket_len)

        tix = np.full((128, RT * (S // 16)), PADIDX, np.uint16)
        tix[16 * gcs + slot % 16, rts * (S // 16) + slot // 16] = jj.astype(np.uint16)
        tgid = np.zeros((128, RT, S), np.float32)
        tgid[16 * gcs + po, rts, slot] = gsel + 1.0
        tgid = tgid.astype(BF16)
        wm = np.zeros((128, RT, S), np.float32)
        wm[16 * gcs + po, rts, slot] = ww
        wm = wm.astype(BF16)

        # head shard + link columns, transposed
        hslice = head_weight[c * SH_SHARD:(c + 1) * SH_SHARD].T.astype(np.float32)
        wheadt = np.concatenate([hslice, linkT], axis=1).astype(BF16)
        wout_t = [
            np.ascontiguousarray(
                outs[g][c * CL_SHARD[g]:(c + 1) * CL_SHARD[g]].T).astype(BF16)
            for g in range(4)
        ]
        in_maps.append({
            "xt": xt,
            "projt": projt,
            "wheadt": wheadt,
            "wout0t": wout_t[0],
            "wout1t": wout_t[1],
            "wout2t": wout_t[2],
            "wout3t": wout_t[3],
            "tgtidx": tix,
            "tgid": tgid,
            "wm": wm,
        })
    return in_maps, S


def _run(features, head_weight, proj0, out0, proj1, out1, proj2, out2,
         proj3, out3, discard_probs, targets, target_mask,
         trace=False, tmpdir=None):
    features = np.asarray(features, np.float32)
    head_weight = np.asarray(head_weight, np.float32)
    projs = [np.asarray(p, np.float32) for p in (proj0, proj1, proj2, proj3)]
    outs = [np.asarray(o, np.float32) for o in (out0, out1, out2, out3)]
    discard_probs = np.asarray(discard_probs, np.float32)

    in_maps, S = _shard_inputs(features, head_weight, projs, outs,
                               discard_probs, targets, target_mask)
    nc = _build(S)
    res = run_bass_kernel_spmd(nc, in_maps, list(range(NCORES)),
                               trace=trace, tmpdir=tmpdir)
    val = np.asarray(res.results[0]["out"], np.float32).reshape(())
    return val, res


def kernel(**inputs) -> np.ndarray:
    val, _ = _run(**inputs)
    return val


# revision 67
# speedup vs baseline: 1.1370x; 1.1370x over previous
"""Adaptive-softmax loss (nn_AdaptiveLoss) on 8 trn2 NeuronCores.

Strategy: tensor-parallel over the vocab dimension, 8-way. Each core owns
1/8 of the shortlist head columns and 1/8 of each tail cluster's output
rows. Per core:

  - computes cluster hidden states h_g = x @ proj_g.T (replicated, small),
  - computes its slice of every group's logits with bf16 matmuls
    (out = lhsT.T @ rhs with lhsT = x.T / h.T tiles shipped pre-transposed),
  - exp()s the logits on the scalar engine (partial softmax denominators
    Z_g[b] accumulate on ACT for the tail clusters and on DVE for the
    shortlist head),
  - gathers exp(logit) at this core's share of the targets straight out of
    SBUF (gpsimd indirect_copy; indices + per-target loss weights are
    routed host-side as part of sharding),
  - reduces per-row partial loss statistics (numerator, weight sums, Z),
  - one 45KB AllReduce combines the statistics, then every core finishes
    the (cheap) log/normalize arithmetic identically and writes the scalar.

The full [B, VOCAB] log-prob matrix is never materialized anywhere.
"""

import sys

sys.path.insert(0, "/opt/trn_rl_repo")

from contextlib import ExitStack

import ml_dtypes
import numpy as np

import concourse.bass as bass  # noqa: F401  (engine types via nc.*)
import concourse.mybir as mybir
import concourse.tile as tile
from concourse import bacc
from concourse.bass_utils import run_bass_kernel_spmd

BF16 = ml_dtypes.bfloat16
F32 = mybir.dt.float32
BF16_DT = mybir.dt.bfloat16
U16 = mybir.dt.uint16

NCORES = 8
B, T, D = 1024, 128, 1024
VOCAB, SHORT = 100000, 10000
CL_SIZES = [10000, 20000, 40000, 20000]
CL_D = [512, 256, 128, 64]
SH_SHARD = SHORT // NCORES                      # 1250
CL_SHARD = [s // NCORES for s in CL_SIZES]      # 1250 2500 5000 2500
GRP_BOUNDS = [0, 10000, 20000, 40000, 80000, 100000]
GRP_SHARD = [SH_SHARD] + CL_SHARD

# per-core concatenated logits layout: [head | links(4) | c0 | c1 | c2 | c3 | pad]
OFF_HEAD = 0
OFF_LINK = SH_SHARD                              # 1250
OFF_CL = [1254, 2504, 5004, 10004]
GRP_OFF = [OFF_HEAD] + OFF_CL                    # per-group concat offset
CONCAT = OFF_CL[-1] + CL_SHARD[-1]               # 12504
CONCAT_PAD = 12544
# pad slots gather column 0 (always computed, finite); their tgid==0 makes
# the ownership mask zero the contribution.
PADIDX = 0
RT = 8                                           # row tiles of 128


# ----------------------------------------------------------------------------
# device kernel builder
# ----------------------------------------------------------------------------

_CACHE: dict[int, object] = {}


def _build(S: int):
    """Build + compile the SPMD kernel for padded slot count S (multiple of 16)."""
    if S in _CACHE:
        return _CACHE[S]
    SW = S // 16

    nc = bacc.Bacc("TRN2", target_bir_lowering=False, debug=False,
                   num_devices=NCORES)

    xt_d = nc.dram_tensor("xt", [D, B], BF16_DT, kind="ExternalInput")
    projt_d = nc.dram_tensor("projt", [D, sum(CL_D)], BF16_DT, kind="ExternalInput")
    whead_d = nc.dram_tensor("wheadt", [D, 1254], BF16_DT, kind="ExternalInput")
    wout_d = [
        nc.dram_tensor(f"wout{g}t", [CL_D[g], CL_SHARD[g]], BF16_DT,
                       kind="ExternalInput")
        for g in range(4)
    ]
    tix_d = nc.dram_tensor("tgtidx", [128, RT * SW], U16, kind="ExternalInput")
    tgid_d = nc.dram_tensor("tgid", [128, RT, S], BF16_DT, kind="ExternalInput")
    wm_d = nc.dram_tensor("wm", [128, RT, S], BF16_DT, kind="ExternalInput")
    out_d = nc.dram_tensor("out", [1, 1], F32, kind="ExternalOutput")

    EXP = mybir.ActivationFunctionType.Exp
    LN = mybir.ActivationFunctionType.Ln
    ADD = mybir.AluOpType.add
    SUB = mybir.AluOpType.subtract
    MULT = mybir.AluOpType.mult
    ISEQ = mybir.AluOpType.is_equal
    ISGE = mybir.AluOpType.is_ge
    AXX = mybir.AxisListType.X

    with tile.TileContext(nc) as tc, ExitStack() as ctx:
        sb = ctx.enter_context(tc.tile_pool(name="sb", bufs=1))
        big = ctx.enter_context(tc.tile_pool(name="big", bufs=3))
        ps = ctx.enter_context(tc.tile_pool(name="ps", bufs=2, space="PSUM"))
        dr = ctx.enter_context(tc.tile_pool(name="dr", bufs=1, space="DRAM"))

        # ---- persistent SBUF tensors ----
        xt_sb = sb.tile([128, 8, B], BF16_DT)          # x.T  [d, b] k-tiled
        whead_sb = sb.tile([128, 8, 1254], BF16_DT)
        wout_sb = [
            sb.tile([CL_D[g] if CL_D[g] < 128 else 128,
                     max(1, CL_D[g] // 128), CL_SHARD[g]], BF16_DT,
                    name=f"wout{g}_sb")
            for g in range(4)
        ]
        h_sb = sb.tile([128, 8, B], BF16_DT)           # h.T tiles (c3 rows 0:64 of ht7)
        tmpS = sb.tile([128, S], BF16_DT)
        tix_sb = sb.tile([128, RT * SW], U16)
        tgid_sb = sb.tile([128, RT, S], BF16_DT)
        vg3 = sb.tile([128, RT, S], BF16_DT)           # gathered exp(logit)
        wm_sb = sb.tile([128, RT, S], BF16_DT)         # (1-dp)*ownership
        logv3 = sb.tile([128, RT, S], BF16_DT)
        linkexp = sb.tile([128, RT, 4], F32)
        zscr = sb.tile([128, 1280], BF16_DT)
        zs7 = sb.tile([128, RT, 7], F32)   # Z partials per piece, c1/c2/c3
        # AR payload, 11 stats x 8 row-tiles:
        # q: 0 Zh, 1..4 Zc_g, 5 den, 6..9 Wg, 10 numraw
        pay = sb.tile([128, 88], F32)
        rsb = sb.tile([128, 88], F32)
        ones_sb = sb.tile([128, 1], F32)
        out_sb = sb.tile([1, 1], F32)

        pview = pay[:, :].rearrange("p (q r) -> p q r", q=11)
        rview = rsb[:, :].rearrange("p (q r) -> p q r", q=11)

        # ---- input DMAs (order matters: compute-critical tensors first;
        # xt/projt interleaved per k-tile so the h matmuls start early) ----
        projt_sb = big.tile([128, 8 * sum(CL_D)], BF16_DT, tag="big")
        pj = projt_sb[:, :].rearrange("p (k c) -> p k c", k=8)
        xt_r = xt_d.ap().rearrange("(k p) b -> p k b", p=128)
        pj_r = projt_d.ap().rearrange("(k p) c -> p k c", p=128)
        for k in range(8):
            nc.sync.dma_start(out=xt_sb[:, k, :], in_=xt_r[:, k, :])
            nc.sync.dma_start(out=pj[:, k, :], in_=pj_r[:, k, :])
        nc.sync.dma_start(out=whead_sb,
                          in_=whead_d.ap().rearrange("(k p) c -> p k c", p=128))
        for g in range(4):
            prt = min(128, CL_D[g])
            nc.sync.dma_start(
                out=wout_sb[g],
                in_=wout_d[g].ap().rearrange("(k p) c -> p k c", p=prt))
        nc.sync.dma_start(out=tix_sb, in_=tix_d[:])
        nc.sync.dma_start(out=tgid_sb, in_=tgid_d[:])
        nc.sync.dma_start(out=wm_sb, in_=wm_d[:])

        nc.vector.memset(ones_sb[:, :], 1.0)

        # ---- cluster hidden states h.T (all batch rows, computed locally —
        # a sharded h + AllGather is infeasible: the collectives entry
        # barrier isn't done until ~60-150us into the NEFF) ----
        HT_OFF = [0, 128, 256, 384, 512, 640, 768, 896]
        HT_M = [128, 128, 128, 128, 128, 128, 128, 64]
        for bc in range(2):
            for htile in range(2):
                pst = ps.tile([128, 2048], F32, tag="ps", name=f"hps_{bc}_{htile}")
                for hl in range(4):
                    ht = htile * 4 + hl
                    M = HT_M[ht]
                    for k in range(8):
                        nc.tensor.matmul(
                            pst[0:M, hl * 512:(hl + 1) * 512],
                            pj[:, k, HT_OFF[ht]:HT_OFF[ht] + M],
                            xt_sb[:, k, bc * 512:(bc + 1) * 512],
                            start=(k == 0), stop=(k == 7))
                for hl in range(4):
                    ht = htile * 4 + hl
                    M = HT_M[ht]
                    nc.vector.tensor_copy(
                        h_sb[0:M, ht, bc * 512:(bc + 1) * 512],
                        pst[0:M, hl * 512:(hl + 1) * 512])

        # ---- main loop: logits -> exp (+Z accumulate) -> gather, per row tile --
        # groups: (concat_off, width, n_ktiles, lhsT source, rhs tensor)
        # lhsT(k, rt): [K, 128] slice; rhs(k, a, w): [K, w] slice
        def lh_head(k, rt):
            return xt_sb[:, k, rt * 128:(rt + 1) * 128]

        def mk_lh_cl(ht0, prt):
            def f(k, rt):
                return h_sb[0:prt, ht0 + k, rt * 128:(rt + 1) * 128]
            return f

        def mk_rhs(tile_):
            def f(k, a, w):
                return tile_[:, k, a:a + w]
            return f

        GROUPS = [
            (0, 1254, 8, lh_head, mk_rhs(whead_sb)),
            (OFF_CL[0], 1250, 4, mk_lh_cl(0, 128), mk_rhs(wout_sb[0])),
            (OFF_CL[1], 2500, 2, mk_lh_cl(4, 128), mk_rhs(wout_sb[1])),
            (OFF_CL[2], 5000, 1, mk_lh_cl(6, 128), mk_rhs(wout_sb[2])),
            (OFF_CL[3], 2500, 1, mk_lh_cl(7, 64), mk_rhs(wout_sb[3])),
        ]

        # Per-group piece lists: (gi, piece_idx, piece_off, piece_width)
        def group_pieces(gi):
            width = GROUPS[gi][1]
            out, a, pi = [], 0, 0
            while a < width:
                w = min(2048, width - a)
                out.append((gi, pi, a, w))
                a += w
                pi += 1
            return out

        # Emission order interleaves the low-K (ACT-bound, PE-light) c2/c3
        # pieces between the K-heavy head/c0/c1 pieces so the PE never
        # drains the 2-deep PSUM pipeline while ACT catches up.
        P_HEAD, P_C0 = group_pieces(0), group_pieces(1)
        P_C1, P_C2, P_C3 = group_pieces(2), group_pieces(3), group_pieces(4)
        PLAN = [P_HEAD[0], P_C2[0], P_C0[0], P_C2[1], P_C1[0], P_C2[2],
                P_C1[1], P_C3[0], P_C3[1]]
        # fixed Z-partial slot per (group, piece), independent of order
        ZMAP = {(2, 0): 0, (2, 1): 1, (3, 0): 2, (3, 1): 3, (3, 2): 4,
                (4, 0): 5, (4, 1): 6}

        for rt in range(RT):
            expb = big.tile([128, CONCAT_PAD], BF16_DT, tag="big",
                            name=f"expb_{rt}")
            for gi, pi, poff, pw in PLAN:
                goff, width, kt, lh, rh = GROUPS[gi]
                pst = ps.tile([128, 2048], F32, tag="ps",
                              name=f"ps_{rt}_{gi}_{pi}")
                # k outer so the stationary operand is loaded once per
                # k-tile instead of once per matmul
                subs = []
                a = 0
                while a < pw:
                    w = min(512, pw - a)
                    subs.append((a, w))
                    a += w
                for k in range(kt):
                    for a, w in subs:
                        nc.tensor.matmul(
                            pst[:, a:a + w],
                            lh(k, rt), rh(k, poff + a, w),
                            start=(k == 0), stop=(k == kt - 1))
                if gi >= 2:
                    # cluster 1/2/3 pieces are single-group: fold the Z
                    # partial into the exp call on ACT
                    nc.scalar.activation(
                        expb[:, goff + poff:goff + poff + pw],
                        pst[:, 0:pw], EXP,
                        accum_out=zs7[:, rt, ZMAP[(gi, pi)]:ZMAP[(gi, pi)] + 1])
                else:
                    nc.scalar.activation(
                        expb[:, goff + poff:goff + poff + pw],
                        pst[:, 0:pw], EXP)
            # head/c0 Z partial sums on DVE (head must exclude link columns)
            for q, (za, zw) in enumerate(
                    [(0, SH_SHARD), (OFF_CL[0], CL_SHARD[0])]):
                nc.vector.tensor_scalar(
                    zscr[:, 0:zw],
                    expb[:, za:za + zw], 1.0, 0.0, op0=MULT, op1=ADD,
                    accum_out=pview[:, q, rt:rt + 1])
            # raw exp(link logits) for the final combine (on ACT so the DVE
            # stats backlog cannot delay releasing this expbuf slot)
            nc.scalar.copy(linkexp[:, rt, :], expb[:, OFF_LINK:OFF_LINK + 4])
            # gather exp(logit) at this core's targets
            nc.gpsimd.indirect_copy(
                vg3[:, rt, :], expb[:, :],
                tix_sb[:, rt * SW:(rt + 1) * SW], True)
            # per-row-tile statistics that don't need log(v): den, W_g
            # (Wsl is never needed: den already equals Wsl + sum_g Wg)
            nc.vector.tensor_reduce(
                pview[:, 5, rt:rt + 1], wm_sb[:, rt, :], AXX, ADD)
            for gi in range(1, 5):
                nc.vector.scalar_tensor_tensor(
                    tmpS[:, :], tgid_sb[:, rt, :], float(gi + 1), wm_sb[:, rt, :],
                    op0=ISEQ, op1=MULT)
                nc.vector.tensor_reduce(
                    pview[:, 5 + gi, rt:rt + 1], tmpS[:, :], AXX, ADD)

        # combine the c1/c2/c3 Z piece-partials into the payload
        t8z = sb.tile([128, 8], F32)
        nc.vector.tensor_tensor(pview[:, 2, :], zs7[:, :, 0], zs7[:, :, 1], ADD)
        nc.vector.tensor_tensor(t8z[:, :], zs7[:, :, 2], zs7[:, :, 3], ADD)
        nc.vector.tensor_tensor(pview[:, 3, :], t8z[:, :], zs7[:, :, 4], ADD)
        nc.vector.tensor_tensor(pview[:, 4, :], zs7[:, :, 5], zs7[:, :, 6], ADD)

        # ---- numerator statistic (needs log of gathered exp) ----
        nc.scalar.activation(
            logv3[:, :, :].rearrange("p a b -> p (a b)"),
            vg3[:, :, :].rearrange("p a b -> p (a b)"), LN)
        tmp3 = vg3
        nc.vector.tensor_tensor(tmp3[:, :, :], logv3[:, :, :], wm_sb[:, :, :], MULT)
        nc.vector.tensor_reduce(pview[:, 10, :], tmp3[:, :, :], AXX, ADD)

        # ---- AllReduce the statistics ----
        cc_in = dr.tile([128, 88], F32)
        cc_out = dr.tile([128, 88], F32, addr_space="Shared")
        nc.sync.dma_start(out=cc_in, in_=pay[:, :])
        nc.gpsimd.collective_compute(
            "AllReduce", ADD,
            replica_groups=[list(range(NCORES))],
            ins=[cc_in.opt()], outs=[cc_out.opt()])
        nc.sync.dma_start(out=rsb, in_=cc_out)

        # ---- final combine (identical on every core) ----
        lsum = sb.tile([128, 8], F32)
        zf = sb.tile([128, 8], F32)
        lzh = sb.tile([128, 8], F32)
        lzc = sb.tile([128, 32], F32)
        llink = sb.tile([128, 32], F32)
        s8 = sb.tile([128, 8], F32)
        tA = sb.tile([128, 8], F32)
        num8 = sb.tile([128, 8], F32)
        rden = sb.tile([128, 8], F32)
        pcol = sb.tile([128, 1], F32)

        nc.vector.tensor_reduce(lsum[:, :], linkexp[:, :, :], AXX, ADD)
        nc.vector.tensor_tensor(zf[:, :], rview[:, 0, :], lsum[:, :], ADD)
        nc.scalar.activation(lzh[:, :], zf[:, :], LN)
        nc.scalar.activation(lzc[:, :], rsb[:, 8:40], LN)
        nc.scalar.activation(
            llink[:, :],
            linkexp[:, :, :].rearrange("p a b -> p (a b)"), LN)
        lzc3 = lzc[:, :].rearrange("p (g r) -> p g r", g=4)
        llink3 = llink[:, :].rearrange("p (r g) -> p r g", g=4)
        for g in range(4):
            nc.vector.tensor_tensor(tA[:, :], llink3[:, :, g], lzc3[:, g, :], SUB)
            if g == 0:
                nc.vector.tensor_tensor(s8[:, :], tA[:, :], rview[:, 6 + g, :], MULT)
            else:
                nc.vector.tensor_tensor(tA[:, :], tA[:, :], rview[:, 6 + g, :], MULT)
                nc.vector.tensor_tensor(s8[:, :], s8[:, :], tA[:, :], ADD)
        # num = numraw + s8 - den * logZh
        nc.vector.tensor_tensor(tA[:, :], rview[:, 5, :], lzh[:, :], MULT)
        nc.vector.tensor_tensor(num8[:, :], rview[:, 10, :], tA[:, :], SUB)
        nc.vector.tensor_tensor(num8[:, :], num8[:, :], s8[:, :], ADD)
        nc.vector.reciprocal(rden[:, :], rview[:, 5, :])
        nc.vector.tensor_tensor(num8[:, :], num8[:, :], rden[:, :], MULT)
        nc.vector.tensor_reduce(pcol[:, :], num8[:, :], AXX, ADD)
        psq = ps.tile([1, 1], F32, tag="ps")
        nc.tensor.matmul(psq[0:1, 0:1], pcol[:, 0:1], ones_sb[:, 0:1],
                         start=True, stop=True)
        nc.scalar.mul(out_sb[:, :], psq[0:1, 0:1], -1.0 / (B + 1e-5))
        nc.sync.dma_start(out=out_d[:], in_=out_sb)

    nc.compile()
    _CACHE[S] = nc
    return nc


# ----------------------------------------------------------------------------
# host-side sharding / index routing
# ----------------------------------------------------------------------------


def _shard_inputs(features, head_weight, projs, outs, discard_probs,
                  targets, target_mask):
    """Build the 8 per-core input maps. Returns (in_maps, S)."""
    xt = np.ascontiguousarray(features.T).astype(BF16)
    projt = np.concatenate([p.T for p in projs], axis=1).astype(BF16)
    linkT = head_weight[SHORT:SHORT + 4].T.astype(np.float32)

    tgt = np.asarray(targets).astype(np.int64).reshape(-1)
    msk = np.asarray(target_mask).astype(bool).reshape(-1)
    bb = np.repeat(np.arange(B, dtype=np.int64), T)

    grp = np.digitize(tgt, GRP_BOUNDS[1:-1])          # 0..4 (0 = shortlist)
    u = tgt - np.asarray(GRP_BOUNDS)[grp]
    shard = np.asarray(GRP_SHARD)[grp]
    core = u // shard
    jcat = u % shard + np.asarray(GRP_OFF)[grp]
    wval = (1.0 - discard_probs[tgt]).astype(np.float32)

    rt = bb >> 7
    gc = (bb >> 4) & 7

    # padded slots per (core, rt, gc)
    key_all = ((core * RT + rt) * 8 + gc).astype(np.int64)
    valid = msk
    counts = np.bincount(key_all[valid], minlength=NCORES * RT * 8)
    # multiple of 32 so each row-tile's wrapped idx slice stays 4B-aligned
    S = int(counts.max())
    S = ((S + 31) // 32) * 32

    in_maps = []
    for c in range(NCORES):
        sel = valid & (core == c)
        jj = jcat[sel]
        bsel = bb[sel]
        gsel = grp[sel]
        rts = rt[sel]
        gcs = gc[sel]
        ww = wval[sel]
        po = bsel & 15
        key = rts * 8 + gcs
        order = np.argsort(key, kind="stable")
        jj, bsel, gsel, rts, gcs, po, ww = (a[order] for a in
                                            (jj, bsel, gsel, rts, gcs, po, ww))
        key = key[order]
        # slot within each (rt, gc) bucket
        start_of = np.r_[0, np.flatnonzero(np.diff(key)) + 1]
        bucket_len = np.diff(np.r_[start_of, len(key)])
        slot = np.arange(len(key)) - np.repeat(start_of, bucket_len)

        tix = np.full((128, RT * (S // 16)), PADIDX, np.uint16)
        tix[16 * gcs + slot % 16, rts * (S // 16) + slot // 16] = jj.astype(np.uint16)
        tgid = np.zeros((128, RT, S), np.float32)
        tgid[16 * gcs + po, rts, slot] = gsel + 1.0
        tgid = tgid.astype(BF16)
        wm = np.zeros((128, RT, S), np.float32)
        wm[16 * gcs + po, rts, slot] = ww
        wm = wm.astype(BF16)

        # head shard + link columns, transposed
        hslice = head_weight[c * SH_SHARD:(c + 1) * SH_SHARD].T.astype(np.float32)
        wheadt = np.concatenate([hslice, linkT], axis=1).astype(BF16)
        wout_t = [
            np.ascontiguousarray(
                outs[g][c * CL_SHARD[g]:(c + 1) * CL_SHARD[g]].T).astype(BF16)
            for g in range(4)
        ]
        in_maps.append({
            "xt": xt,
            "projt": projt,
            "wheadt": wheadt,
            "wout0t": wout_t[0],
            "wout1t": wout_t[1],
            "wout2t": wout_t[2],
            "wout3t": wout_t[3],
            "tgtidx": tix,
            "tgid": tgid,
            "wm": wm,
        })
    return in_maps, S


def _run(features, head_weight, proj0, out0, proj1, out1, proj2, out2,
         proj3, out3, discard_probs, targets, target_mask,
         trace=False, tmpdir=None):
    features = np.asarray(features, np.float32)
    head_weight = np.asarray(head_weight, np.float32)
    projs = [np.asarray(p, np.float32) for p in (proj0, proj1, proj2, proj3)]
    outs = [np.asarray(o, np.float32) for o in (out0, out1, out2, out3)]
    discard_probs = np.asarray(discard_probs, np.float32)

    in_maps, S = _shard_inputs(features, head_weight, projs, outs,
                               discard_probs, targets, target_mask)
    nc = _build(S)
    res = run_bass_kernel_spmd(nc, in_maps, list(range(NCORES)),
                               trace=trace, tmpdir=tmpdir)
    val = np.asarray(res.results[0]["out"], np.float32).reshape(())
    return val, res


def kernel(**inputs) -> np.ndarray:
    val, _ = _run(**inputs)
    return val
